# revision 12
# baseline (speedup 1.0000x reference)
"""Chamfer loss kernel for Trainium2 (8 NeuronCores).

loss = 0.5*(mean_i sqrt(min_j ||t_i-o_j||^2) + mean_j sqrt(min_i ||o_j-t_i||^2))
       * 10 / 1.02**(cur//20)

Strategy
--------
Both NN searches are sharded over the query-point dimension across the 8
cores.  Queries are grouped into 128-row KD-tree leaves (recursive median
split -> compact boxes).  For each leaf the host computes a rigorous
per-row NN upper bound ub_r (min of the generating-pair distance and the
exact best among +-256 Morton-rank candidate neighbours), then gathers
exactly the candidates inside the union of balls B(row_r, ub_r) — the
minimal certified set: every row's true NN provably lies in it, so the
device window-min IS the global min.

Tiles are split into chunks of <= 1024 candidates, all 2-direction
chunks are pooled, sorted by width and dealt in groups of 8 to the
cores, so all cores execute the identical static slot schedule (SPMD)
and are load-balanced by construction.  Host combines duplicate-row
slot results with min.

On device, per slot: one (or two) matmuls with the K=18 bf16 hi/lo
expansion of the homogeneous distance form emit complete squared
distances (negated) to PSUM; per slot either the vector engine reduces
straight from PSUM, or the scalar engine drains PSUM to fp16 and one
vector tensor_tensor_reduce folds+reduces — chosen greedily to balance
the two engines.  Candidate data is prefetched with a handful of large
DMAs.  Device outputs per-row max(-d) = -min d.
"""

import numpy as np

N = 32768
NCORES = 8
LEAF = 128                 # query rows per tile (one stationary group)
CAP = 1024                 # max candidate columns per slot (one PSUM tile)
GRAN = 64                  # slot width granularity
SENT = 100.0               # sentinel coordinate for padding
K = 18                     # contraction rows of the bf16 hi/lo expansion
UBWIN = 512                # half-window (in Morton ranks) for the ub bound
MMW = 512                  # matmul moving-operand width (PSUM bank limit)
PIECE = 4096               # candidate-DMA piece size (columns)
PACKW = 256                # slots this wide or less get packed reduces

_cached = {}


# ----------------------------------------------------------------- device

def _build_program(widths):
    import concourse.bacc as bacc
    import concourse.tile as tile
    from concourse import mybir

    f32 = mybir.dt.float32
    f16 = mybir.dt.float16
    bf16 = mybir.dt.bfloat16
    nc = bacc.Bacc("TRN2", target_bir_lowering=False, debug=False)

    ns = len(widths)
    tot = sum(widths)
    lhs = nc.dram_tensor("lhs", (K, ns * 128), bf16, kind="ExternalInput")
    cand = nc.dram_tensor("cand", (K, tot), bf16, kind="ExternalInput")
    res = nc.dram_tensor("res", (128, ns), f32, kind="ExternalOutput")

    # group consecutive slots into DMA pieces of <= PIECE columns
    pieces = []          # (col_start, col_end)
    slot_piece = []      # slot -> (piece index, offset inside piece)
    off = 0
    pstart = 0
    for j, w in enumerate(widths):
        if off + w - pstart > PIECE and off > pstart:
            pieces.append((pstart, off))
            pstart = off
        slot_piece.append((len(pieces), off - pstart))
        off += w
    pieces.append((pstart, off))

    with tile.TileContext(nc) as tc:
        with (
            tc.tile_pool(name="lhs", bufs=1) as lhs_pool,
            tc.tile_pool(name="cand", bufs=1) as cand_pool,
            tc.tile_pool(name="acc", bufs=1) as acc_pool,
            tc.tile_pool(name="stage", bufs=3) as stage_pool,
            tc.tile_pool(name="junk", bufs=2) as junk_pool,
            tc.tile_pool(name="ps", bufs=4, space="PSUM") as ps_pool,
        ):
            lhs_sb = lhs_pool.tile([K, ns * 128], bf16, tag="lhs")
            nc.sync.dma_start(out=lhs_sb, in_=lhs[:])
            ptiles = []
            for p, (a, b) in enumerate(pieces):
                pt = cand_pool.tile([K, b - a], bf16, tag=f"c{p}", name=f"c{p}")
                # piece 0 gates the first matmul: issue it on the sync
                # queue right after lhs; later pieces go via the otherwise
                # idle gpsimd queue so descriptor setup runs in parallel
                eng = nc.sync if p == 0 else nc.gpsimd
                eng.dma_start(out=pt, in_=cand[:, a:b])
                ptiles.append(pt)
            racc = acc_pool.tile([128, ns], f32, tag="racc")

            def emit_mms(ps, pcol, j, w):
                """All matmuls for slot j into ps starting at column pcol."""
                p, poff = slot_piece[j]
                pt = ptiles[p]
                lhsT = lhs_sb[:, j * 128:(j + 1) * 128]
                for j0 in range(0, w, MMW):
                    n = min(MMW, w - j0)
                    nc.tensor.matmul(
                        ps[:, pcol + j0:pcol + j0 + n],
                        lhsT,
                        pt[:, poff + j0:poff + j0 + n],
                        start=True,
                        stop=True,
                    )

            def reduce_a(ps, j, k, w):
                """DVE reduces straight from PSUM (1x, ~1.2 ns/col)."""
                if k == 1:
                    in_ = ps[:, :w]
                else:
                    in_ = ps[:, :k * w].rearrange("p (k w) -> p k w", w=w)
                nc.vector.tensor_reduce(
                    out=racc[:, j:j + k], in_=in_,
                    axis=mybir.AxisListType.X, op=mybir.AluOpType.max,
                )

            def reduce_b(ps, j, k, w):
                """ACT drains PSUM to fp16; DVE folds 2x then reduces w/4."""
                kw = k * w
                h, q = w // 2, w // 4
                s = stage_pool.tile([128, CAP], f16, tag="s")
                nc.scalar.copy(out=s[:, :kw], in_=ps[:, :kw])
                f1 = junk_pool.tile([128, CAP // 2], f16, tag="f1")
                f2 = junk_pool.tile([128, CAP // 4], f16, tag="f2")
                if k == 1:
                    nc.vector.tensor_max(f1[:, :h], s[:, :h], s[:, h:w])
                    nc.vector.tensor_max(f2[:, :q], f1[:, :q], f1[:, q:h])
                    in_ = f2[:, :q]
                else:
                    sv = s[:, :kw].rearrange("p (k w) -> p k w", w=w)
                    f1v = f1[:, :kw // 2].rearrange("p (k h) -> p k h", h=h)
                    nc.vector.tensor_max(f1v, sv[:, :, :h], sv[:, :, h:])
                    f2v = f2[:, :kw // 4].rearrange("p (k q) -> p k q", q=q)
                    nc.vector.tensor_max(f2v, f1v[:, :, :q], f1v[:, :, q:])
                    in_ = f2v
                nc.vector.tensor_reduce(
                    out=racc[:, j:j + k], in_=in_,
                    axis=mybir.AxisListType.X, op=mybir.AluOpType.max,
                )

            act_ns = 0.0
            dve_ns = 0.0
            j = 0
            while j < ns:
                w = widths[j]
                k = 1
                if w <= PACKW and 512 % w == 0:
                    # pack a run of equal-width small slots into one PSUM
                    # tile; segmented (3D-AP) ops cover the whole pack
                    while (j + k < ns and widths[j + k] == w
                           and (k + 1) * w <= CAP):
                        k += 1
                ps = ps_pool.tile([128, CAP], f32, tag="ps")
                for i in range(k):
                    emit_mms(ps, i * w, j + i, w)
                kw = k * w
                ca_d = 1.20 * kw + 90.0
                cb_a = 1.05 * kw + 150.0
                cb_d = 0.63 * kw + 390.0
                if max(act_ns, dve_ns + ca_d) <= max(act_ns + cb_a,
                                                     dve_ns + cb_d):
                    dve_ns += ca_d
                    reduce_a(ps, j, k, w)
                else:
                    act_ns += cb_a
                    dve_ns += cb_d
                    reduce_b(ps, j, k, w)
                j += k
            nc.sync.dma_start(out=res[:], in_=racc)

    nc.compile()
    return nc


def _get_program(widths):
    key = widths
    if key not in _cached:
        _cached[key] = _build_program(widths)
    return _cached[key]


# ------------------------------------------------------------------- host

def _bf16():
    import ml_dtypes
    return ml_dtypes.bfloat16


def _split2(v32):
    bf = _bf16()
    hi = v32.astype(bf)
    lo = (v32 - hi.astype(np.float32)).astype(bf)
    return hi, lo


def _split3(v64):
    bf = _bf16()
    a = v64.astype(np.float32).astype(bf)
    r = v64 - a.astype(np.float64)
    b = r.astype(np.float32).astype(bf)
    r = r - b.astype(np.float64)
    c = r.astype(np.float32).astype(bf)
    return a, b, c


def _pack(points):
    """[n,3] -> (lhs rows [K,n], cand rows [K,n]) in bf16 such that
    lhsT.T @ cand accumulates the squared distance d = |q|^2+|c|^2-2q.c
    to ~1e-7 via hi/lo splits.  Row pairing k: lhs[k]*cand[k]:
      0-2 qh*(-2ch)  3-5 ql*(-2ch)  6-8 qh*(-2cl)  9-11 ql*(-2cl)
      12-14 q2(3-way)*1   15-17 1*c2(3-way)
    """
    bf = _bf16()
    n = points.shape[0]
    xh, xl = _split2(points.T.astype(np.float32))
    q64 = xh.astype(np.float64) + xl.astype(np.float64)
    p2 = (q64 * q64).sum(0)
    p2a, p2b, p2c = _split3(p2)

    L = np.empty((K, n), bf)
    L[0:3] = xh
    L[3:6] = xl
    L[6:9] = xh
    L[9:12] = xl
    L[12] = p2a
    L[13] = p2b
    L[14] = p2c
    L[15:18] = np.ones((3, n), bf)

    R = np.empty((K, n), bf)
    m2h = (-2.0 * xh.astype(np.float32)).astype(bf)
    m2l = (-2.0 * xl.astype(np.float32)).astype(bf)
    R[0:3] = m2h
    R[3:6] = m2h
    R[6:9] = m2l
    R[9:12] = m2l
    R[12:15] = np.ones((3, n), bf)
    R[15] = p2a
    R[16] = p2b
    R[17] = p2c
    return L, R


def _morton(pts):
    q = np.clip((pts / 1.1 * 1024).astype(np.int64), 0, 1023)

    def spread(v):
        v = (v | (v << 16)) & 0x030000FF
        v = (v | (v << 8)) & 0x0300F00F
        v = (v | (v << 4)) & 0x030C30C3
        v = (v | (v << 2)) & 0x09249249
        return v

    return (spread(q[:, 0]) << 2) | (spread(q[:, 1]) << 1) | spread(q[:, 2])


def _ub_bound(rows, cands, pair_ub):
    """Rigorous per-row upper bound on the NN distance: min of the
    generating-pair distance and the exact best among +-UBWIN
    Morton-rank candidate neighbours (f32 eval, inflated for rounding)."""
    n = len(rows)
    co = np.argsort(_morton(cands), kind="stable")
    cs = cands[co].astype(np.float32)
    cms = _morton(cands)[co]
    pos = np.searchsorted(cms, _morton(rows))
    ub = np.empty(n, np.float64)
    win = np.arange(-UBWIN, UBWIN)
    rs32 = rows.astype(np.float32)
    for s in range(0, n, 2048):
        e = min(s + 2048, n)
        idx = np.clip(pos[s:e, None] + win[None, :], 0, n - 1)
        d = ((rs32[s:e, None, :] - cs[idx]) ** 2).sum(-1)
        ub[s:e] = d.min(1)
    ub = np.sqrt(ub) * 1.00001 + 1e-7          # cover f32 rounding
    return np.minimum(ub, pair_ub)


def _kd_tiles(pts):
    """Recursive median split -> index arrays of size LEAF (compact boxes)."""
    out = []

    def rec(idx):
        if len(idx) == LEAF:
            out.append(idx)
            return
        p = pts[idx]
        dim = int(np.argmax(p.max(0) - p.min(0)))
        k = len(idx) // 2
        part = np.argpartition(p[:, dim], k)
        rec(idx[part[:k]])
        rec(idx[part[k:]])

    rec(np.arange(len(pts)))
    return out


def _tile_slots(rows, cands, ubd, d):
    """KD-tile the queries, gather the minimal certified candidate set per
    tile (union of per-row ub balls), chunk to <= CAP.  Returns slot list
    [(nominal_width, dir, row_idx[LEAF], cand_idx_chunk)]."""
    rows64 = rows.astype(np.float64)
    cands64 = cands.astype(np.float64)
    c2 = (cands64 * cands64).sum(-1)
    slots = []
    for ti in _kd_tiles(rows64):
        blk = rows64[ti]
        ub = ubd[ti]
        R = ub.max()
        lo = blk.min(0) - R
        hi = blk.max(0) + R
        ci = np.flatnonzero(((cands64 >= lo) & (cands64 <= hi)).all(1))
        # exact refine: keep c iff some row's ub ball contains it
        d2 = (c2[ci][:, None] + (blk * blk).sum(-1)[None, :]
              - 2.0 * (cands64[ci] @ blk.T))
        ci = ci[(d2 <= (ub * ub)[None, :] + 1e-9).any(1)]
        nch = max(1, -(-len(ci) // CAP))
        for chunk in np.array_split(ci, nch):
            slots.append((len(chunk), d, ti, chunk))
    return slots


def _prep(target, output, pair_ub):
    """Returns (widths, in_maps, rmaps): device slot widths, per-core
    input arrays, and per-core [(slot, dir, rows)] result maps."""
    bf = _bf16()
    ub1 = _ub_bound(target, output, pair_ub)
    ub2 = _ub_bound(output, target, pair_ub)
    slots = (_tile_slots(target, output, ub1, 0)
             + _tile_slots(output, target, ub2, 1))
    order = sorted(range(len(slots)), key=lambda i: -slots[i][0])
    ns = -(-len(order) // NCORES)

    widths = []
    for j in range(ns):
        g = order[j * NCORES:(j + 1) * NCORES]
        wmax = max(slots[i][0] for i in g)
        widths.append(max(GRAN, -(-wmax // GRAN) * GRAN))
    widths = tuple(widths)
    tot = sum(widths)

    L1, _ = _pack(target)
    _, R1 = _pack(output)
    L2, _ = _pack(output)
    _, R2 = _pack(target)
    L1 = (-L1.astype(np.float32)).astype(bf)   # PE emits -d
    L2 = (-L2.astype(np.float32)).astype(bf)
    sentL, sentR = _pack(np.full((1, 3), SENT, np.float32))
    sentL = (-sentL.astype(np.float32)).astype(bf)
    Ls = (L1, L2)
    Rs = (R1, R2)

    in_maps = []
    rmaps = []
    for c in range(NCORES):
        lhs_m = np.empty((K, ns * 128), bf)
        lhs_m[:] = sentL
        cand_m = np.empty((K, tot), bf)
        cand_m[:] = sentR
        rmap = []
        off = 0
        for j in range(ns):
            gi = j * NCORES + c
            if gi < len(order):
                _, d, ti, chunk = slots[order[gi]]
                lhs_m[:, j * 128:(j + 1) * 128] = Ls[d][:, ti]
                cand_m[:, off:off + len(chunk)] = Rs[d][:, chunk]
                rmap.append((j, d, ti))
            off += widths[j]
        in_maps.append({"lhs": lhs_m, "cand": cand_m})
        rmaps.append(rmap)
    return widths, in_maps, rmaps


def _install_ntff_hook_shim():
    """The agent image's `antenv` lacks `axon_hooks`, which bass_utils
    imports unconditionally when trace=True under axon.  Provide it,
    wired to the ctypes NTFF profiler from trn_agent_boot."""
    import sys, types
    if "antenv.axon_hooks" in sys.modules:
        return
    hook = None
    try:
        from trn_agent_boot.trn_boot import _ntff_profile_via_ctypes
        hook = _ntff_profile_via_ctypes("/opt/axon/libaxon_pjrt.so")
    except Exception:
        pass
    mod = types.ModuleType("antenv.axon_hooks")
    mod._hook = hook
    mod.get_axon_ntff_profile_hook = lambda: mod._hook

    def set_axon_ntff_profile_hook(h):
        mod._hook = h

    mod.set_axon_ntff_profile_hook = set_axon_ntff_profile_hook
    sys.modules["antenv.axon_hooks"] = mod
    try:
        import antenv
        antenv.axon_hooks = mod
    except Exception:
        pass


def _run(target, output, cur, trace=False):
    if trace:
        _install_ntff_hook_shim()
    from concourse.bass_utils import run_bass_kernel_spmd

    target = np.asarray(target, np.float32)
    output = np.asarray(output, np.float32)
    pair_ub = np.sqrt(
        ((target.astype(np.float64) - output.astype(np.float64)) ** 2).sum(-1)
    ) * 1.0000001

    widths, in_maps, rmaps = _prep(target, output, pair_ub)
    nc = _get_program(widths)
    r = run_bass_kernel_spmd(nc, in_maps, core_ids=list(range(NCORES)),
                             trace=trace)

    mins = [np.full(N, np.inf), np.full(N, np.inf)]
    for c in range(NCORES):
        blk = np.asarray(r.results[c]["res"], np.float64)   # [128, ns]
        for j, d, ti in rmaps[c]:
            np.minimum.at(mins[d], ti, -blk[:, j])
    m1 = np.maximum(mins[0], 0.0)
    m2 = np.maximum(mins[1], 0.0)
    loss = 0.5 * (np.sqrt(m1).mean() + np.sqrt(m2).mean())
    loss = loss * 10.0 / (1.02 ** (int(cur) // 20))
    return np.float32(loss), r


def kernel(target, output, cur):
    out, _ = _run(target, output, cur)
    return out


# revision 15
# speedup vs baseline: 1.0898x; 1.0898x over previous
"""Chamfer loss kernel for Trainium2 (8 NeuronCores).

loss = 0.5*(mean_i sqrt(min_j ||t_i-o_j||^2) + mean_j sqrt(min_i ||o_j-t_i||^2))
       * 10 / 1.02**(cur//20)

Strategy
--------
Both NN searches are sharded over the query-point dimension across the 8
cores.  Queries are grouped into 128-row KD-tree leaves (recursive median
split -> compact boxes).  For each leaf the host computes a rigorous
per-row NN upper bound ub_r (min of the generating-pair distance and the
exact best among +-256 Morton-rank candidate neighbours), then gathers
exactly the candidates inside the union of balls B(row_r, ub_r) — the
minimal certified set: every row's true NN provably lies in it, so the
device window-min IS the global min.

Tiles are split into chunks of <= 1024 candidates, all 2-direction
chunks are pooled, sorted by width and dealt in groups of 8 to the
cores, so all cores execute the identical static slot schedule (SPMD)
and are load-balanced by construction.  Host combines duplicate-row
slot results with min.

On device, per slot: one (or two) matmuls with the K=18 bf16 hi/lo
expansion of the homogeneous distance form emit complete squared
distances (negated) to PSUM; per slot either the vector engine reduces
straight from PSUM, or the scalar engine drains PSUM to fp16 and one
vector tensor_tensor_reduce folds+reduces — chosen greedily to balance
the two engines.  Candidate data is prefetched with a handful of large
DMAs.  Device outputs per-row max(-d) = -min d.
"""

import numpy as np

N = 32768
NCORES = 8
LEAF = 128                 # query rows per tile (one stationary group)
CAP = 1024                 # max candidate columns per slot (one PSUM tile)
GRAN = 64                  # slot width granularity
SENT = 100.0               # sentinel coordinate for padding
K = 18                     # contraction rows of the bf16 hi/lo expansion
UBWIN = 512                # half-window (in Morton ranks) for the ub bound
MMW = 512                  # matmul moving-operand width (PSUM bank limit)
PIECE = 8192               # candidate-DMA piece size (columns)
PACKW = 256                # slots this wide or less get packed reduces

_cached = {}


# ----------------------------------------------------------------- device

def _build_program(widths):
    import concourse.bacc as bacc
    import concourse.tile as tile
    from concourse import mybir

    f32 = mybir.dt.float32
    f16 = mybir.dt.float16
    bf16 = mybir.dt.bfloat16
    nc = bacc.Bacc("TRN2", target_bir_lowering=False, debug=False)

    ns = len(widths)
    tot = sum(widths)
    lhs = nc.dram_tensor("lhs", (K, ns * 128), bf16, kind="ExternalInput")
    cand = nc.dram_tensor("cand", (K, tot), bf16, kind="ExternalInput")
    res = nc.dram_tensor("res", (128, ns), f32, kind="ExternalOutput")

    # group consecutive slots into DMA pieces: a small first piece so the
    # first matmul can start early, larger ones after
    pieces = []          # (col_start, col_end)
    slot_piece = []      # slot -> (piece index, offset inside piece)
    off = 0
    pstart = 0
    for j, w in enumerate(widths):
        cap = 2048 if not pieces else PIECE
        if off + w - pstart > cap and off > pstart:
            pieces.append((pstart, off))
            pstart = off
        slot_piece.append((len(pieces), off - pstart))
        off += w
    pieces.append((pstart, off))

    with tile.TileContext(nc) as tc:
        with (
            tc.tile_pool(name="lhs", bufs=1) as lhs_pool,
            tc.tile_pool(name="cand", bufs=1) as cand_pool,
            tc.tile_pool(name="acc", bufs=1) as acc_pool,
            tc.tile_pool(name="stage", bufs=3) as stage_pool,
            tc.tile_pool(name="junk", bufs=2) as junk_pool,
            tc.tile_pool(name="ps", bufs=4, space="PSUM") as ps_pool,
        ):
            lhs_sb = lhs_pool.tile([K, ns * 128], bf16, tag="lhs")
            nc.sync.dma_start(out=lhs_sb, in_=lhs[:])
            ptiles = []
            for p, (a, b) in enumerate(pieces):
                pt = cand_pool.tile([K, b - a], bf16, tag=f"c{p}", name=f"c{p}")
                nc.sync.dma_start(out=pt, in_=cand[:, a:b])
                ptiles.append(pt)
            racc = acc_pool.tile([128, ns], f32, tag="racc")

            def emit_mms(ps, pcol, j, w):
                """All matmuls for slot j into ps starting at column pcol."""
                p, poff = slot_piece[j]
                pt = ptiles[p]
                lhsT = lhs_sb[:, j * 128:(j + 1) * 128]
                for j0 in range(0, w, MMW):
                    n = min(MMW, w - j0)
                    nc.tensor.matmul(
                        ps[:, pcol + j0:pcol + j0 + n],
                        lhsT,
                        pt[:, poff + j0:poff + j0 + n],
                        start=True,
                        stop=True,
                    )

            def reduce_a(ps, j, k, w):
                """DVE reduces straight from PSUM (1x, ~1.2 ns/col)."""
                if k == 1:
                    in_ = ps[:, :w]
                else:
                    in_ = ps[:, :k * w].rearrange("p (k w) -> p k w", w=w)
                nc.vector.tensor_reduce(
                    out=racc[:, j:j + k], in_=in_,
                    axis=mybir.AxisListType.X, op=mybir.AluOpType.max,
                )

            def reduce_b(ps, j, k, w):
                """ACT drains PSUM to fp16; DVE folds 2x then reduces w/4."""
                kw = k * w
                h, q = w // 2, w // 4
                s = stage_pool.tile([128, CAP], f16, tag="s")
                nc.scalar.copy(out=s[:, :kw], in_=ps[:, :kw])
                f1 = junk_pool.tile([128, CAP // 2], f16, tag="f1")
                f2 = junk_pool.tile([128, CAP // 4], f16, tag="f2")
                if k == 1:
                    nc.vector.tensor_max(f1[:, :h], s[:, :h], s[:, h:w])
                    nc.vector.tensor_max(f2[:, :q], f1[:, :q], f1[:, q:h])
                    in_ = f2[:, :q]
                else:
                    sv = s[:, :kw].rearrange("p (k w) -> p k w", w=w)
                    f1v = f1[:, :kw // 2].rearrange("p (k h) -> p k h", h=h)
                    nc.vector.tensor_max(f1v, sv[:, :, :h], sv[:, :, h:])
                    f2v = f2[:, :kw // 4].rearrange("p (k q) -> p k q", q=q)
                    nc.vector.tensor_max(f2v, f1v[:, :, :q], f1v[:, :, q:])
                    in_ = f2v
                nc.vector.tensor_reduce(
                    out=racc[:, j:j + k], in_=in_,
                    axis=mybir.AxisListType.X, op=mybir.AluOpType.max,
                )

            act_ns = 0.0
            dve_ns = 0.0
            j = 0
            while j < ns:
                w = widths[j]
                k = 1
                if w <= PACKW and 512 % w == 0:
                    # pack a run of equal-width small slots into one PSUM
                    # tile; segmented (3D-AP) ops cover the whole pack
                    while (j + k < ns and widths[j + k] == w
                           and (k + 1) * w <= CAP):
                        k += 1
                ps = ps_pool.tile([128, CAP], f32, tag="ps")
                for i in range(k):
                    emit_mms(ps, i * w, j + i, w)
                kw = k * w
                ca_d = 1.20 * kw + 90.0
                cb_a = 1.05 * kw + 150.0
                cb_d = 0.63 * kw + 390.0
                if max(act_ns, dve_ns + ca_d) <= max(act_ns + cb_a,
                                                     dve_ns + cb_d):
                    dve_ns += ca_d
                    reduce_a(ps, j, k, w)
                else:
                    act_ns += cb_a
                    dve_ns += cb_d
                    reduce_b(ps, j, k, w)
                j += k
            nc.sync.dma_start(out=res[:], in_=racc)

    nc.compile()
    return nc


def _get_program(widths):
    key = widths
    if key not in _cached:
        _cached[key] = _build_program(widths)
    return _cached[key]


# ------------------------------------------------------------------- host

def _bf16():
    import ml_dtypes
    return ml_dtypes.bfloat16


def _split2(v32):
    bf = _bf16()
    hi = v32.astype(bf)
    lo = (v32 - hi.astype(np.float32)).astype(bf)
    return hi, lo


def _split3(v64):
    bf = _bf16()
    a = v64.astype(np.float32).astype(bf)
    r = v64 - a.astype(np.float64)
    b = r.astype(np.float32).astype(bf)
    r = r - b.astype(np.float64)
    c = r.astype(np.float32).astype(bf)
    return a, b, c


def _pack(points):
    """[n,3] -> (lhs rows [K,n], cand rows [K,n]) in bf16 such that
    lhsT.T @ cand accumulates the squared distance d = |q|^2+|c|^2-2q.c
    to ~1e-7 via hi/lo splits.  Row pairing k: lhs[k]*cand[k]:
      0-2 qh*(-2ch)  3-5 ql*(-2ch)  6-8 qh*(-2cl)  9-11 ql*(-2cl)
      12-14 q2(3-way)*1   15-17 1*c2(3-way)
    """
    bf = _bf16()
    n = points.shape[0]
    xh, xl = _split2(points.T.astype(np.float32))
    q64 = xh.astype(np.float64) + xl.astype(np.float64)
    p2 = (q64 * q64).sum(0)
    p2a, p2b, p2c = _split3(p2)

    L = np.empty((K, n), bf)
    L[0:3] = xh
    L[3:6] = xl
    L[6:9] = xh
    L[9:12] = xl
    L[12] = p2a
    L[13] = p2b
    L[14] = p2c
    L[15:18] = np.ones((3, n), bf)

    R = np.empty((K, n), bf)
    m2h = (-2.0 * xh.astype(np.float32)).astype(bf)
    m2l = (-2.0 * xl.astype(np.float32)).astype(bf)
    R[0:3] = m2h
    R[3:6] = m2h
    R[6:9] = m2l
    R[9:12] = m2l
    R[12:15] = np.ones((3, n), bf)
    R[15] = p2a
    R[16] = p2b
    R[17] = p2c
    return L, R


def _morton(pts):
    q = np.clip((pts / 1.1 * 1024).astype(np.int64), 0, 1023)

    def spread(v):
        v = (v | (v << 16)) & 0x030000FF
        v = (v | (v << 8)) & 0x0300F00F
        v = (v | (v << 4)) & 0x030C30C3
        v = (v | (v << 2)) & 0x09249249
        return v

    return (spread(q[:, 0]) << 2) | (spread(q[:, 1]) << 1) | spread(q[:, 2])


def _ub_bound(rows, cands, pair_ub):
    """Rigorous per-row upper bound on the NN distance: min of the
    generating-pair distance and the exact best among +-UBWIN
    Morton-rank candidate neighbours (f32 eval, inflated for rounding)."""
    n = len(rows)
    co = np.argsort(_morton(cands), kind="stable")
    cs = cands[co].astype(np.float32)
    cms = _morton(cands)[co]
    pos = np.searchsorted(cms, _morton(rows))
    ub = np.empty(n, np.float64)
    win = np.arange(-UBWIN, UBWIN)
    rs32 = rows.astype(np.float32)
    for s in range(0, n, 2048):
        e = min(s + 2048, n)
        idx = np.clip(pos[s:e, None] + win[None, :], 0, n - 1)
        d = ((rs32[s:e, None, :] - cs[idx]) ** 2).sum(-1)
        ub[s:e] = d.min(1)
    ub = np.sqrt(ub) * 1.00001 + 1e-7          # cover f32 rounding
    return np.minimum(ub, pair_ub)


def _kd_tiles(pts):
    """Recursive median split -> index arrays of size LEAF (compact boxes)."""
    out = []

    def rec(idx):
        if len(idx) == LEAF:
            out.append(idx)
            return
        p = pts[idx]
        dim = int(np.argmax(p.max(0) - p.min(0)))
        k = len(idx) // 2
        part = np.argpartition(p[:, dim], k)
        rec(idx[part[:k]])
        rec(idx[part[k:]])

    rec(np.arange(len(pts)))
    return out


def _tile_slots(rows, cands, ubd, d):
    """KD-tile the queries, gather the minimal certified candidate set per
    tile (union of per-row ub balls), chunk to <= CAP.  Returns slot list
    [(nominal_width, dir, row_idx[LEAF], cand_idx_chunk)]."""
    rows64 = rows.astype(np.float64)
    cands64 = cands.astype(np.float64)
    c2 = (cands64 * cands64).sum(-1)
    slots = []
    for ti in _kd_tiles(rows64):
        blk = rows64[ti]
        ub = ubd[ti]
        R = ub.max()
        lo = blk.min(0) - R
        hi = blk.max(0) + R
        ci = np.flatnonzero(((cands64 >= lo) & (cands64 <= hi)).all(1))
        # exact refine: keep c iff some row's ub ball contains it
        d2 = (c2[ci][:, None] + (blk * blk).sum(-1)[None, :]
              - 2.0 * (cands64[ci] @ blk.T))
        ci = ci[(d2 <= (ub * ub)[None, :] + 1e-9).any(1)]
        nch = max(1, -(-len(ci) // CAP))
        for chunk in np.array_split(ci, nch):
            slots.append((len(chunk), d, ti, chunk))
    return slots


def _prep(target, output, pair_ub):
    """Returns (widths, in_maps, rmaps): device slot widths, per-core
    input arrays, and per-core [(slot, dir, rows)] result maps."""
    bf = _bf16()
    ub1 = _ub_bound(target, output, pair_ub)
    ub2 = _ub_bound(output, target, pair_ub)
    slots = (_tile_slots(target, output, ub1, 0)
             + _tile_slots(output, target, ub2, 1))
    order = sorted(range(len(slots)), key=lambda i: -slots[i][0])
    ns = -(-len(order) // NCORES)

    widths = []
    for j in range(ns):
        g = order[j * NCORES:(j + 1) * NCORES]
        wmax = max(slots[i][0] for i in g)
        widths.append(max(GRAN, -(-wmax // GRAN) * GRAN))
    widths = tuple(widths)
    tot = sum(widths)

    L1, _ = _pack(target)
    _, R1 = _pack(output)
    L2, _ = _pack(output)
    _, R2 = _pack(target)
    L1 = (-L1.astype(np.float32)).astype(bf)   # PE emits -d
    L2 = (-L2.astype(np.float32)).astype(bf)
    sentL, sentR = _pack(np.full((1, 3), SENT, np.float32))
    sentL = (-sentL.astype(np.float32)).astype(bf)
    Ls = (L1, L2)
    Rs = (R1, R2)

    in_maps = []
    rmaps = []
    for c in range(NCORES):
        lhs_m = np.empty((K, ns * 128), bf)
        lhs_m[:] = sentL
        cand_m = np.empty((K, tot), bf)
        cand_m[:] = sentR
        rmap = []
        off = 0
        for j in range(ns):
            gi = j * NCORES + c
            if gi < len(order):
                _, d, ti, chunk = slots[order[gi]]
                lhs_m[:, j * 128:(j + 1) * 128] = Ls[d][:, ti]
                cand_m[:, off:off + len(chunk)] = Rs[d][:, chunk]
                rmap.append((j, d, ti))
            off += widths[j]
        in_maps.append({"lhs": lhs_m, "cand": cand_m})
        rmaps.append(rmap)
    return widths, in_maps, rmaps


def _install_ntff_hook_shim():
    """The agent image's `antenv` lacks `axon_hooks`, which bass_utils
    imports unconditionally when trace=True under axon.  Provide it,
    wired to the ctypes NTFF profiler from trn_agent_boot."""
    import sys, types
    if "antenv.axon_hooks" in sys.modules:
        return
    hook = None
    try:
        from trn_agent_boot.trn_boot import _ntff_profile_via_ctypes
        hook = _ntff_profile_via_ctypes("/opt/axon/libaxon_pjrt.so")
    except Exception:
        pass
    mod = types.ModuleType("antenv.axon_hooks")
    mod._hook = hook
    mod.get_axon_ntff_profile_hook = lambda: mod._hook

    def set_axon_ntff_profile_hook(h):
        mod._hook = h

    mod.set_axon_ntff_profile_hook = set_axon_ntff_profile_hook
    sys.modules["antenv.axon_hooks"] = mod
    try:
        import antenv
        antenv.axon_hooks = mod
    except Exception:
        pass


def _run(target, output, cur, trace=False):
    if trace:
        _install_ntff_hook_shim()
    from concourse.bass_utils import run_bass_kernel_spmd

    target = np.asarray(target, np.float32)
    output = np.asarray(output, np.float32)
    pair_ub = np.sqrt(
        ((target.astype(np.float64) - output.astype(np.float64)) ** 2).sum(-1)
    ) * 1.0000001

    widths, in_maps, rmaps = _prep(target, output, pair_ub)
    nc = _get_program(widths)
    r = run_bass_kernel_spmd(nc, in_maps, core_ids=list(range(NCORES)),
                             trace=trace)

    mins = [np.full(N, np.inf), np.full(N, np.inf)]
    for c in range(NCORES):
        blk = np.asarray(r.results[c]["res"], np.float64)   # [128, ns]
        for j, d, ti in rmaps[c]:
            np.minimum.at(mins[d], ti, -blk[:, j])
    m1 = np.maximum(mins[0], 0.0)
    m2 = np.maximum(mins[1], 0.0)
    loss = 0.5 * (np.sqrt(m1).mean() + np.sqrt(m2).mean())
    loss = loss * 10.0 / (1.02 ** (int(cur) // 20))
    return np.float32(loss), r


def kernel(target, output, cur):
    out, _ = _run(target, output, cur)
    return out
